# revision 61
# baseline (speedup 1.0000x reference)
"""Trainium2 Bass kernel for nn_ConvNat (2x NeighborhoodAttention2D + dwconv + linear).

v2 strategy (8 NeuronCores, SPMD), dense-masked attention:
  - Multiplicative bias: host precomputes expB = exp(rpb) in-window, 0 outside.
    P = exp(S) * expB, applied on the vector engine (bf16 2x) -- removes the
    bias identity-matmuls (7128 PE cols/layer) from the tensor engine.
  - Merged-head S matmuls: block-diagonal zero-padded Q [64, 4*Q] so one
    [64,128] stationary computes all 4 heads' S for a key chunk in 2 matmuls
    (512+136 cols) instead of 4.
  - exp in single 648-col slices straight from PSUM.
  - dwconv tap matmuls + layer-2 q-proj run inside the AllGather bubble;
    b1 table DMA'd in 3 pieces so chunk 0 starts early.
"""

import numpy as np
import ml_dtypes

BF16 = ml_dtypes.bfloat16

HEADS = 4
K = 31
C = 64
DH = 16
H = W = 36
N = H * W            # 1296 tokens
NCORES = 8
Q = N // NCORES      # 162 queries per core
NCH = 11             # n-chunks of 128 (1296 -> 1408 padded)
NPAD = NCH * 128
GC = HEADS * Q       # 648 cols per key-chunk block (h, q)
PTW = NCH * GC       # 7128
N_PRE = 0            # PE warm-up dummy matmuls before layer 1
N_BUB = 0            # PE keep-warm dummy matmuls through the AllGather bubble

_CACHE = {}


def _build_bias(rpb, t0):
    """Multiplicative bias exp(rpb) in-window else 0, for queries [t0,t0+Q).
    Layout (128, NCH, HEADS, Q) -> (128, PTW): col = nb*648 + h*162 + q."""
    n = np.arange(NPAD)
    r = np.minimum(n // 36, 35)
    c = n % 36
    valid_n = (n < N)
    t = np.arange(t0, t0 + Q)
    i = t // 36
    j = t % 36
    ri = np.clip(i - K // 2, 0, H - K)
    cj = np.clip(j - K // 2, 0, W - K)
    rm = (r[:, None] >= ri[None, :]) & (r[:, None] <= ri[None, :] + K - 1)
    cm = (c[:, None] >= cj[None, :]) & (c[:, None] <= cj[None, :] + K - 1)
    m = rm & cm & valid_n[:, None]       # (NPAD, Q)
    rrel = np.clip(r[:, None] - i[None, :] + (K - 1), 0, 2 * K - 2)
    crel = np.clip(c[:, None] - j[None, :] + (K - 1), 0, 2 * K - 2)
    bias = rpb[:, rrel, crel]            # (HEADS, NPAD, Q)
    bias = np.where(m[None], np.exp(bias), 0.0).astype(np.float32)
    bias = bias.reshape(HEADS, NCH, 128, Q).transpose(2, 1, 0, 3)
    return np.ascontiguousarray(bias.reshape(128, PTW))


def _interleaved_wk(qkv_w, qkv_b, off, scale=1.0):
    """[65, 128] stationary: cols 32h+0..16 = head-h rows (off+16h+d) of qkv_w^T."""
    wt = np.zeros((C + 1, 128), np.float32)
    for h in range(HEADS):
        rows = qkv_w[off + 16 * h: off + 16 * h + 16]
        wt[:C, 32 * h: 32 * h + 16] = rows.T * scale
        wt[C, 32 * h: 32 * h + 16] = qkv_b[off + 16 * h: off + 16 * h + 16] * scale
    return wt


def _interleaved_wk128(eff_w, eff_b, off, scale=1.0):
    """[128, 128] stationary over the 32-row-per-head gathered layout
    (row 32g+d = channel 16g+d for d<16; row 16 = the ones/bias row)."""
    wt = np.zeros((128, 128), np.float32)
    for kh in range(HEADS):
        rT = eff_w[off + 16 * kh: off + 16 * kh + 16].T  # [C, 16]
        for g in range(HEADS):
            wt[32 * g:32 * g + 16, 32 * kh:32 * kh + 16] = \
                rT[16 * g:16 * g + 16] * scale
        wt[16, 32 * kh:32 * kh + 16] = \
            eff_b[off + 16 * kh: off + 16 * kh + 16] * scale
    return wt


def _wv128(eff_w, eff_b):
    """[128, C] v-projection stationary over the gathered layout."""
    wt = np.zeros((128, C), np.float32)
    wT = eff_w[2 * C:].T  # [C, C]
    for g in range(HEADS):
        wt[32 * g:32 * g + 16] = wT[16 * g:16 * g + 16]
    wt[16] += eff_b[2 * C:]
    return wt


def _bf(a):
    return np.ascontiguousarray(np.asarray(a, np.float32).astype(BF16))


def _prep(inputs):
    x = np.asarray(inputs['x'], np.float32).reshape(N, C)
    p = {}
    xT = np.zeros((C + 1, NPAD), np.float32)
    xT[:C, :N] = x.T
    xT[C, :N] = 1.0
    p['xT'] = xT
    scale = DH ** -0.5
    proj_w1 = np.asarray(inputs['proj_w1'], np.float32)
    proj_b1 = np.asarray(inputs['proj_b1'], np.float32)
    qkv_w1 = np.asarray(inputs['qkv_w1'], np.float32)
    qkv_b1 = np.asarray(inputs['qkv_b1'], np.float32)
    p['wq1'] = _interleaved_wk(qkv_w1, qkv_b1, 0, scale)
    p['wk1'] = _interleaved_wk(qkv_w1, qkv_b1, C)
    wv1 = np.zeros((C + 1, C), np.float32)
    wv1[:C] = qkv_w1[2 * C:].T
    wv1[C] = qkv_b1[2 * C:]
    p['wv1'] = wv1
    # layer-2 qkv with proj1 folded in (y1 = proj1 @ attn1 + proj_b1)
    qkv_w2 = np.asarray(inputs['qkv_w2'], np.float32)
    qkv_b2 = np.asarray(inputs['qkv_b2'], np.float32)
    eff_w2 = qkv_w2 @ proj_w1
    eff_b2 = qkv_w2 @ proj_b1 + qkv_b2
    p['wq2'] = _interleaved_wk(qkv_w2, qkv_b2, 0, scale)
    p['wk2'] = _interleaved_wk(qkv_w2, qkv_b2, C)
    wv2 = np.zeros((C + 1, C), np.float32)
    wv2[:C] = qkv_w2[2 * C:].T
    wv2[C] = qkv_b2[2 * C:]
    p['wv2'] = wv2
    pr1 = np.zeros((C + 1, C), np.float32)
    pr1[:C] = proj_w1.T
    pr1[C] = proj_b1
    lin_w = np.asarray(inputs['lin_w'], np.float32)
    lin_b = np.asarray(inputs['lin_b'], np.float32)
    proj_w2 = np.asarray(inputs['proj_w2'], np.float32)
    proj_b2 = np.asarray(inputs['proj_b2'], np.float32)
    dw_w = np.asarray(inputs['dw_w'], np.float32)               # (64, 1, 3, 3)
    dw_b = np.asarray(inputs['dw_b'], np.float32)
    m2 = np.zeros((C + 1, C), np.float32)
    m2[:C] = (lin_w @ proj_w2).T
    m2[C] = lin_w @ proj_b2 + lin_w @ dw_b + lin_b
    mtap = np.zeros((9, C, C), np.float32)
    for di in range(3):
        for dj in range(3):
            mtap[di * 3 + dj] = (lin_w * dw_w[None, :, 0, di, dj]).T
    # pack small stationary tensors into one [128, 1472] tensor (one DMA)
    wpack = np.zeros((128, 1472), np.float32)
    p['wq1e'] = p['wq1']
    p['wk1e'] = p['wk1']
    p['wv1e'] = p['wv1']
    wpack[:C + 1, 0:128] = p.pop('wq1')
    wpack[:C + 1, 128:256] = p.pop('wk1')
    wpack[:C + 1, 256:384] = p.pop('wq2')
    wpack[:C + 1, 384:512] = p.pop('wk2')
    wpack[:C + 1, 512:576] = p.pop('wv1')
    wpack[:C + 1, 576:640] = p.pop('wv2')
    wpack[:C + 1, 640:704] = pr1
    wpack[:C + 1, 704:768] = m2
    wpack[:, 768:896] = np.eye(128, dtype=np.float32)
    wpack[:C, 896:1472] = mtap.transpose(1, 0, 2).reshape(C, 576)
    p['wpack'] = wpack
    # per-core tensors
    x_img = x.reshape(H, W, C).transpose(2, 0, 1)
    xpad = np.zeros((C, H + 2, W + 2), np.float32)
    xpad[:, 1:-1, 1:-1] = x_img
    rpb1 = np.asarray(inputs['rpb1'], np.float32)
    rpb2 = np.asarray(inputs['rpb2'], np.float32)
    percore = []
    for core in range(NCORES):
        t0 = core * Q
        d = {}
        # aux packs per-core xq ([65,Q] at cols 0:Q) and xdw ([C,9Q] at
        # cols Q:Q+9Q, tap-major) into one tensor so xq can be a tiny
        # first DMA piece
        aux = np.zeros((128, Q + 9 * Q), np.float32)
        aux[:C, 0:Q] = x[t0:t0 + Q].T
        aux[C, 0:Q] = 1.0
        for di in range(3):
            for dj in range(3):
                sh = xpad[:, di:di + H, dj:dj + W].reshape(C, N)
                t = di * 3 + dj
                aux[:C, Q + Q * t:Q + Q * t + Q] = sh[:, t0:t0 + Q]
        d['aux'] = aux
        d['b1'] = _build_bias(rpb1, t0)
        d['b2'] = _build_bias(rpb2, t0)
        percore.append(d)
    p = {k: _bf(v) for k, v in p.items()}
    percore = [{k: _bf(v) for k, v in d.items()} for d in percore]
    return p, percore


def _build_program():
    import concourse.bass as bass
    import concourse.bacc as bacc
    import concourse.tile as tile
    from concourse import mybir
    f32 = mybir.dt.float32
    bf16 = mybir.dt.bfloat16
    AF = mybir.ActivationFunctionType

    nc = bacc.Bacc("TRN2", target_bir_lowering=False, debug=False,
                   num_devices=NCORES)

    di = {}
    for name, shape in [
        ('xT', [C + 1, NPAD]), ('aux', [128, 10 * Q]),
        ('wq1e', [C + 1, 128]), ('wk1e', [C + 1, 128]),
        ('wv1e', [C + 1, C]), ('wpack', [128, 1472]),
        ('b1', [128, PTW]), ('b2', [128, PTW]),
    ]:
        di[name] = nc.dram_tensor(name, shape, bf16, kind="ExternalInput")
    out_d = nc.dram_tensor('out', [C, Q], f32, kind="ExternalOutput")
    cc_in = nc.dram_tensor('cc_in', [C, Q], bf16)
    cc_out = nc.dram_tensor('cc_out', [NCORES, C, Q], bf16,
                            addr_space="Shared")

    with tile.TileContext(nc) as tc:
        with (
            tc.tile_pool(name="const", bufs=1) as cpool,
            tc.tile_pool(name="work", bufs=2) as wpool,
            tc.tile_pool(name="ps_big", bufs=3, space="PSUM") as psb,
            tc.tile_pool(name="ps_held", bufs=1, space="PSUM") as psh,
        ):
            # ---- PE clock warm-up (HAM): dummy matmuls on memset data ----
            warm_src = cpool.tile([128, 512], bf16, name='warm_src')
            nc.vector.memset(warm_src[:], 0.125)
            warm_ctr = [0]

            def warm_mm(n):
                if n <= 0:
                    return
                warm_ctr[0] += 1
                ps_dummy = psb.tile([128, 1024], f32,
                                    name=f'ps_dummy{warm_ctr[0]}', tag='big')
                for _ in range(n):
                    nc.tensor.matmul(ps_dummy[:16, :512], warm_src[:, 0:16],
                                     warm_src[:], start=True, stop=True,
                                     skip_group_check=True)

            warm_mm(N_PRE)
            # ---- constant loads ----
            # duplicate wq1 as its own tiny first DMA so the q-proj
            # LDWEIGHTS isn't gated on the full wpack transfer
            wq1_sb = cpool.tile([C + 1, 128], bf16, name='wq1_sb')
            nc.sync.dma_start(wq1_sb[:], di['wq1e'][:])
            xqT = cpool.tile([C + 1, Q], bf16, name='xqT')
            nc.sync.dma_start(xqT[:], di['aux'][0:C + 1, 0:Q])
            # xT in two tiles so k-proj chunk 0 / early v-proj aren't gated
            # on the full transfer (whole-tile dependency)
            wk1_sb = cpool.tile([C + 1, 128], bf16, name='wk1_sb')
            nc.sync.dma_start(wk1_sb[:], di['wk1e'][:])
            xTa = cpool.tile([C + 1, 512], bf16, name='xTa')
            nc.sync.dma_start(xTa[:], di['xT'][:, 0:512])
            wv1_sb = cpool.tile([C + 1, C], bf16, name='wv1_sb')
            nc.sync.dma_start(wv1_sb[:], di['wv1e'][:])
            wpack_sb = cpool.tile([128, 1472], bf16, name='wpack_sb')
            nc.sync.dma_start(wpack_sb[:], di['wpack'][:])
            xTb = cpool.tile([C + 1, NPAD - 512], bf16, name='xTb')
            nc.sync.dma_start(xTb[:], di['xT'][:, 512:NPAD])
            b_sb = {}
            for l in (1, 2):
                b_sb[l] = cpool.tile([128, PTW], bf16, name=f'b{l}_sb')
            # b1 in 3 pieces so chunk 0 can start early
            for s0, s1 in ((0, 4 * GC), (4 * GC, 8 * GC), (8 * GC, PTW)):
                nc.sync.dma_start(b_sb[1][:, s0:s1], di['b1'][:, s0:s1])
            xdw_sb = cpool.tile([C, 9 * Q], bf16, name='xdw_sb')
            nc.sync.dma_start(xdw_sb[:], di['aux'][0:C, Q:10 * Q])
            for s0, s1 in ((0, 6 * GC), (6 * GC, PTW)):
                nc.sync.dma_start(b_sb[2][:, s0:s1], di['b2'][:, s0:s1])
            w_sb = {
                'wq1': wq1_sb[:, :],
                'wk1': wk1_sb[:, :],
                'wq2': wpack_sb[0:C + 1, 256:384],
                'wk2': wpack_sb[0:C + 1, 384:512],
                'wv1': wv1_sb[:, :],
                'wv2': wpack_sb[0:C + 1, 576:640],
                'proj1': wpack_sb[0:C + 1, 640:704],
                'm2p': wpack_sb[0:C + 1, 704:768],
            }
            id_sb = wpack_sb[0:128, 768:896]
            mtap_sb = wpack_sb[0:C, 896:1472]

            # preload exp table early
            dummy = cpool.tile([1, 1], f32, name='dummy')
            nc.vector.memset(dummy[:], 0.0)
            dummy2 = cpool.tile([1, 1], f32, name='dummy2')
            nc.scalar.activation(dummy2[:], dummy[:], AF.Exp)

            x2T = cpool.tile([C + 1, NPAD], bf16, name='x2T')
            nc.vector.memset(x2T[:, N:], 0.0)
            nc.vector.memset(x2T[C:C + 1, :N], 1.0)
            y1T = cpool.tile([C + 1, Q], bf16, name='y1T')
            nc.vector.memset(y1T[C:C + 1, :], 1.0)

            def nat_layer(l, srcT, src_qT, after_pv=None, srcT2=None):
                """srcT: [65, NPAD] AP (or cols 0:512 when srcT2 covers
                512:NPAD); src_qT: [65, Q] AP. -> attnT [65, Q]."""
                wq, wk, wv = w_sb[f'wq{l}'], w_sb[f'wk{l}'], w_sb[f'wv{l}']

                def src(s0, sz):
                    if srcT2 is not None and s0 >= 512:
                        return srcT2[:, s0 - 512:s0 - 512 + sz]
                    return srcT[:, s0:s0 + sz]
                # q proj -> Mq block-diagonal [64, 648]
                ps_q = psb.tile([128, 1024], f32, name='ps_q', tag='big')
                nc.tensor.matmul(ps_q[:, :Q], wq, src_qT, start=True, stop=True)
                Mq = wpool.tile([128, GC], bf16, name='Mq')
                nc.vector.memset(Mq[:], 0.0)
                for h in range(HEADS):
                    nc.vector.tensor_copy(Mq[32 * h:32 * h + 16, Q * h:Q * h + Q],
                                          ps_q[32 * h:32 * h + 16, :Q])
                # k proj chunk 0 first (unblocks S chunk 0), then v proj
                # (keeps the PE busy while k casts run), then k tail
                kT = wpool.tile([128, NPAD], bf16, name='kT')
                nc.vector.memset(kT[:, N:], 0.0)
                VV = wpool.tile([128, NCH * 68], bf16, name='VV')
                VVr = VV[:].rearrange("p (nb g d) -> p nb g d", g=HEADS, d=17)
                nc.vector.memset(VV[:], 0.0)
                nc.vector.memset(VVr[:, :, :, 16:17], 1.0)

                def kproj(s0, sz):
                    ps_k = psb.tile([128, 1024], f32, name='ps_k', tag='big')
                    nc.tensor.matmul(ps_k[:, :sz], wk, src(s0, sz),
                                     start=True, stop=True)
                    nc.vector.tensor_copy(kT[:, s0:s0 + sz], ps_k[:, :sz])

                def vproj(nb):
                    nv = 128 if nb < NCH - 1 else N - 128 * (NCH - 1)
                    ps_v = psb.tile([128, 1024], f32, name='ps_v', tag='big')
                    nc.tensor.matmul(ps_v[:nv, :C],
                                     src(128 * nb, nv),
                                     wv, start=True, stop=True)
                    nc.vector.tensor_copy(
                        VVr[:nv, nb, :, 0:16],
                        ps_v[:nv, :C].rearrange("p (g d) -> p g d", d=16))

                kproj(0, 512)
                for nb in range(NCH):
                    vproj(nb)
                kproj(512, 512)
                kproj(1024, 272)
                # chunk pipeline: S (2 mm) -> exp -> mult -> PV (accum)
                PTr = wpool.tile([128, PTW], bf16, name='PTr')
                PT = wpool.tile([128, PTW], bf16, name='PT')
                ps_o = psh.tile([128, 512], f32, name='ps_o')
                for nb in range(NCH):
                    psS = psb.tile([128, 1024], f32, name='psS', tag='big')
                    kc = kT[:, 128 * nb:128 * nb + 128]
                    nc.tensor.matmul(psS[:, 0:512], kc, Mq[:, 0:512],
                                     start=True, stop=True,
                                     skip_group_check=True)
                    nc.tensor.matmul(psS[:, 512:GC], kc, Mq[:, 512:GC],
                                     start=True, stop=True,
                                     skip_group_check=True)
                    base = GC * nb
                    nc.scalar.activation(PTr[:, base:base + GC],
                                         psS[:, 0:GC], AF.Exp)
                    nc.vector.tensor_mul(PT[:, base:base + GC],
                                         PTr[:, base:base + GC],
                                         b_sb[l][:, base:base + GC])
                    # head-pair PV: one 324-col matmul per pair; out rows
                    # 0:34 (heads 0,1 x query-col blocks) and 64:98 (2,3)
                    for pr in range(2):
                        nc.tensor.matmul(
                            ps_o[64 * pr:64 * pr + 34, 0:2 * Q],
                            VV[:, 68 * nb + 34 * pr:68 * nb + 34 * pr + 34],
                            PT[:, base + 2 * Q * pr:base + 2 * Q * pr + 2 * Q],
                            start=(nb == 0), stop=(nb == NCH - 1),
                            skip_group_check=True, tile_position=(0, 64 * pr))
                if after_pv is not None:
                    after_pv(ps_o)
                # normalize: transpose each pair block -> reciprocal ->
                # scale. After the transpose, channel rows become columns,
                # so the pair layout only changes column indices here.
                o_sbA = wpool.tile([128, Q], bf16, name='o_sbA')
                o_sbB = wpool.tile([128, Q], bf16, name='o_sbB')
                nc.vector.tensor_copy(o_sbA[:], ps_o[:, 0:Q])
                nc.scalar.copy(o_sbB[:], ps_o[:, Q:2 * Q])
                ps_t = psb.tile([128, 2048], bf16, name='ps_t', tag='big')
                nc.tensor.transpose(ps_t[:, 0:128], o_sbA[:, 0:128], id_sb)
                nc.tensor.transpose(ps_t[:34, 256:384], o_sbA[:, 128:Q], id_sb)
                nc.tensor.transpose(ps_t[:, 1024:1152], o_sbB[:, 0:128], id_sb)
                nc.tensor.transpose(ps_t[:34, 1280:1408], o_sbB[:, 128:Q],
                                    id_sb)
                rec = wpool.tile([128, 8], f32, name='rec')
                tA0 = ps_t[:, 0:128].rearrange("p (g d) -> p g d", d=64)
                tB0 = ps_t[:, 1024:1152].rearrange("p (g d) -> p g d", d=64)
                tA1 = ps_t[:34, 256:384].rearrange("p (g d) -> p g d", d=64)
                tB1 = ps_t[:34, 1280:1408].rearrange("p (g d) -> p g d", d=64)
                nc.vector.reciprocal(rec[:, 0:2], tA0[:, :, 16:17])
                nc.vector.reciprocal(rec[:, 2:4], tB0[:, :, 33:34])
                nc.vector.reciprocal(rec[:34, 4:6], tA1[:, :, 16:17])
                nc.vector.reciprocal(rec[:34, 6:8], tB1[:, :, 33:34])
                aq0 = wpool.tile([128, C], bf16, name='aq0')
                aq1 = wpool.tile([34, C], bf16, name='aq1')
                for h in range(HEADS):
                    in_a = (h % 2 == 0)
                    b0 = 0 if in_a else 1024
                    b1 = 256 if in_a else 1280
                    col = 64 * (h // 2) + (0 if in_a else 17)
                    r0 = {0: 0, 2: 1, 1: 2, 3: 3}[h]
                    # split the 8 normalize muls across vector and scalar
                    if h < 2:
                        nc.vector.tensor_scalar_mul(
                            aq0[:, 16 * h:16 * h + 16],
                            ps_t[:, b0 + col:b0 + col + 16],
                            rec[:, r0:r0 + 1])
                        nc.vector.tensor_scalar_mul(
                            aq1[:, 16 * h:16 * h + 16],
                            ps_t[:34, b1 + col:b1 + col + 16],
                            rec[:34, 4 + r0:5 + r0])
                    else:
                        nc.scalar.activation(
                            aq0[:, 16 * h:16 * h + 16],
                            ps_t[:, b0 + col:b0 + col + 16], AF.Copy,
                            scale=rec[:, r0:r0 + 1])
                        nc.scalar.activation(
                            aq1[:, 16 * h:16 * h + 16],
                            ps_t[:34, b1 + col:b1 + col + 16], AF.Copy,
                            scale=rec[:34, 4 + r0:5 + r0])
                ps_a = psb.tile([128, 2048], bf16, name='ps_a', tag='big')
                nc.tensor.transpose(ps_a[:C, 0:128], aq0[:], id_sb)
                nc.tensor.transpose(ps_a[:C, 1024:1058], aq1[:], id_sb[:34, :34])
                attnT = wpool.tile([C + 1, Q], bf16, name=f'attnT{l}')
                nc.vector.memset(attnT[C:C + 1, :], 1.0)
                nc.scalar.copy(attnT[:C, 0:128], ps_a[:C, 0:128])
                nc.vector.tensor_copy(attnT[:C, 128:Q], ps_a[:C, 1024:1058])
                return attnT

            # ---------------- layer 1 ----------------
            attnT1 = nat_layer(1, xTa[:], xqT[:, :], srcT2=xTb[:])
            ps_y = psb.tile([128, 1024], f32, name='ps_y', tag='big')
            nc.tensor.matmul(ps_y[:C, :Q], w_sb['proj1'], attnT1[:],
                             start=True, stop=True)
            nc.scalar.copy(y1T[:C, :], ps_y[:C, :Q])
            nc.sync.dma_start(cc_in[:], y1T[:C, :])
            nc.gpsimd.collective_compute(
                "AllGather", mybir.AluOpType.bypass,
                replica_groups=[list(range(NCORES))],
                ins=[cc_in.ap().opt()], outs=[cc_out.ap().opt()])
            # dwconv taps overlap the collective
            ps_z = psh.tile([128, 512], f32, name='ps_z')
            for t in range(9):
                nc.tensor.matmul(ps_z[:C, :Q],
                                 mtap_sb[:, C * t:C * t + C],
                                 xdw_sb[:, Q * t:Q * t + Q],
                                 start=(t == 0), stop=False,
                                 skip_group_check=True)
            # keep the PE clock warm through the AllGather bubble
            warm_mm(N_BUB)
            nc.sync.dma_start(x2T[:C, :N],
                              cc_out.ap().rearrange("r c q -> c r q"))
            # ---------------- layer 2 ----------------
            attnT2 = nat_layer(2, x2T[:], y1T[:])
            zo = wpool.tile([C, Q], f32, name='zo')
            nc.tensor.matmul(ps_z[:C, 0:128], w_sb['m2p'], attnT2[:, 0:128],
                             start=False, stop=True, skip_group_check=True)
            nc.vector.tensor_copy(zo[:, 0:128], ps_z[:C, 0:128])
            nc.sync.dma_start(out_d[:, 0:128], zo[:, 0:128])
            nc.tensor.matmul(ps_z[:C, 128:Q], w_sb['m2p'], attnT2[:, 128:Q],
                             start=False, stop=True, skip_group_check=True)
            nc.scalar.copy(zo[:, 128:Q], ps_z[:C, 128:Q])
            nc.sync.dma_start(out_d[:, 128:Q], zo[:, 128:Q])

    nc.finalize()
    return nc


def kernel(**inputs) -> np.ndarray:
    from concourse.bass_utils import run_bass_kernel_spmd
    if 'nc' not in _CACHE:
        _CACHE['nc'] = _build_program()
    nc = _CACHE['nc']
    shared, percore = _prep(inputs)
    in_maps = []
    for core in range(NCORES):
        m = dict(shared)
        m.update(percore[core])
        in_maps.append(m)
    res = run_bass_kernel_spmd(nc, in_maps, core_ids=list(range(NCORES)))
    outs = [np.asarray(res.results[c]['out']).T for c in range(NCORES)]
    full = np.concatenate(outs, axis=0).reshape(1, N, C)
    return full.astype(np.float32)


if __name__ == '__main__':
    import reference
    inputs = reference.setup_inputs()
    inputs = {k: np.asarray(v) for k, v in inputs.items()}
    got = kernel(**inputs)
    print("kernel output", got.shape, got.dtype)



# revision 62
# speedup vs baseline: 1.1058x; 1.1058x over previous
"""Trainium2 Bass kernel for nn_ConvNat (2x NeighborhoodAttention2D + dwconv + linear).

v2 strategy (8 NeuronCores, SPMD), dense-masked attention:
  - Multiplicative bias: host precomputes expB = exp(rpb) in-window, 0 outside.
    P = exp(S) * expB, applied on the vector engine (bf16 2x) -- removes the
    bias identity-matmuls (7128 PE cols/layer) from the tensor engine.
  - Merged-head S matmuls: block-diagonal zero-padded Q [64, 4*Q] so one
    [64,128] stationary computes all 4 heads' S for a key chunk in 2 matmuls
    (512+136 cols) instead of 4.
  - exp in single 648-col slices straight from PSUM.
  - dwconv tap matmuls + layer-2 q-proj run inside the AllGather bubble;
    b1 table DMA'd in 3 pieces so chunk 0 starts early.
"""

import numpy as np
import ml_dtypes

BF16 = ml_dtypes.bfloat16

HEADS = 4
K = 31
C = 64
DH = 16
H = W = 36
N = H * W            # 1296 tokens
NCORES = 8
Q = N // NCORES      # 162 queries per core
NCH = 11             # n-chunks of 128 (1296 -> 1408 padded)
NPAD = NCH * 128
GC = HEADS * Q       # 648 cols per key-chunk block (h, q)
PTW = NCH * GC       # 7128
N_PRE = 0            # PE warm-up dummy matmuls before layer 1
N_BUB = 0            # PE keep-warm dummy matmuls through the AllGather bubble

_CACHE = {}


def _build_bias(rpb, t0):
    """Multiplicative bias exp(rpb) in-window else 0, for queries [t0,t0+Q).
    Layout (128, NCH, HEADS, Q) -> (128, PTW): col = nb*648 + h*162 + q."""
    n = np.arange(NPAD)
    r = np.minimum(n // 36, 35)
    c = n % 36
    valid_n = (n < N)
    t = np.arange(t0, t0 + Q)
    i = t // 36
    j = t % 36
    ri = np.clip(i - K // 2, 0, H - K)
    cj = np.clip(j - K // 2, 0, W - K)
    rm = (r[:, None] >= ri[None, :]) & (r[:, None] <= ri[None, :] + K - 1)
    cm = (c[:, None] >= cj[None, :]) & (c[:, None] <= cj[None, :] + K - 1)
    m = rm & cm & valid_n[:, None]       # (NPAD, Q)
    rrel = np.clip(r[:, None] - i[None, :] + (K - 1), 0, 2 * K - 2)
    crel = np.clip(c[:, None] - j[None, :] + (K - 1), 0, 2 * K - 2)
    bias = rpb[:, rrel, crel]            # (HEADS, NPAD, Q)
    bias = np.where(m[None], np.exp(bias), 0.0).astype(np.float32)
    bias = bias.reshape(HEADS, NCH, 128, Q).transpose(2, 1, 0, 3)
    return np.ascontiguousarray(bias.reshape(128, PTW))


def _interleaved_wk(qkv_w, qkv_b, off, scale=1.0):
    """[65, 128] stationary: cols 32h+0..16 = head-h rows (off+16h+d) of qkv_w^T."""
    wt = np.zeros((C + 1, 128), np.float32)
    for h in range(HEADS):
        rows = qkv_w[off + 16 * h: off + 16 * h + 16]
        wt[:C, 32 * h: 32 * h + 16] = rows.T * scale
        wt[C, 32 * h: 32 * h + 16] = qkv_b[off + 16 * h: off + 16 * h + 16] * scale
    return wt


def _interleaved_wk128(eff_w, eff_b, off, scale=1.0):
    """[128, 128] stationary over the 32-row-per-head gathered layout
    (row 32g+d = channel 16g+d for d<16; row 16 = the ones/bias row)."""
    wt = np.zeros((128, 128), np.float32)
    for kh in range(HEADS):
        rT = eff_w[off + 16 * kh: off + 16 * kh + 16].T  # [C, 16]
        for g in range(HEADS):
            wt[32 * g:32 * g + 16, 32 * kh:32 * kh + 16] = \
                rT[16 * g:16 * g + 16] * scale
        wt[16, 32 * kh:32 * kh + 16] = \
            eff_b[off + 16 * kh: off + 16 * kh + 16] * scale
    return wt


def _wv128(eff_w, eff_b):
    """[128, C] v-projection stationary over the gathered layout."""
    wt = np.zeros((128, C), np.float32)
    wT = eff_w[2 * C:].T  # [C, C]
    for g in range(HEADS):
        wt[32 * g:32 * g + 16] = wT[16 * g:16 * g + 16]
    wt[16] += eff_b[2 * C:]
    return wt


def _bf(a):
    return np.ascontiguousarray(np.asarray(a, np.float32).astype(BF16))


def _prep(inputs):
    x = np.asarray(inputs['x'], np.float32).reshape(N, C)
    p = {}
    xT = np.zeros((C + 1, NPAD), np.float32)
    xT[:C, :N] = x.T
    xT[C, :N] = 1.0
    p['xT'] = xT
    scale = DH ** -0.5
    proj_w1 = np.asarray(inputs['proj_w1'], np.float32)
    proj_b1 = np.asarray(inputs['proj_b1'], np.float32)
    qkv_w1 = np.asarray(inputs['qkv_w1'], np.float32)
    qkv_b1 = np.asarray(inputs['qkv_b1'], np.float32)
    p['wq1'] = _interleaved_wk(qkv_w1, qkv_b1, 0, scale)
    p['wk1'] = _interleaved_wk(qkv_w1, qkv_b1, C)
    wv1 = np.zeros((C + 1, C), np.float32)
    wv1[:C] = qkv_w1[2 * C:].T
    wv1[C] = qkv_b1[2 * C:]
    p['wv1'] = wv1
    # layer-2 qkv with proj1 folded in (y1 = proj1 @ attn1 + proj_b1)
    qkv_w2 = np.asarray(inputs['qkv_w2'], np.float32)
    qkv_b2 = np.asarray(inputs['qkv_b2'], np.float32)
    eff_w2 = qkv_w2 @ proj_w1
    eff_b2 = qkv_w2 @ proj_b1 + qkv_b2
    p['wq2'] = _interleaved_wk(qkv_w2, qkv_b2, 0, scale)
    p['wk2'] = _interleaved_wk(qkv_w2, qkv_b2, C)
    wv2 = np.zeros((C + 1, C), np.float32)
    wv2[:C] = qkv_w2[2 * C:].T
    wv2[C] = qkv_b2[2 * C:]
    p['wv2'] = wv2
    pr1 = np.zeros((C + 1, C), np.float32)
    pr1[:C] = proj_w1.T
    pr1[C] = proj_b1
    lin_w = np.asarray(inputs['lin_w'], np.float32)
    lin_b = np.asarray(inputs['lin_b'], np.float32)
    proj_w2 = np.asarray(inputs['proj_w2'], np.float32)
    proj_b2 = np.asarray(inputs['proj_b2'], np.float32)
    dw_w = np.asarray(inputs['dw_w'], np.float32)               # (64, 1, 3, 3)
    dw_b = np.asarray(inputs['dw_b'], np.float32)
    m2 = np.zeros((C + 1, C), np.float32)
    m2[:C] = (lin_w @ proj_w2).T
    m2[C] = lin_w @ proj_b2 + lin_w @ dw_b + lin_b
    mtap = np.zeros((9, C, C), np.float32)
    for di in range(3):
        for dj in range(3):
            mtap[di * 3 + dj] = (lin_w * dw_w[None, :, 0, di, dj]).T
    # pack small stationary tensors into one [128, 1472] tensor (one DMA)
    wpack = np.zeros((128, 1472), np.float32)
    p['wq1e'] = p['wq1']
    p['wk1e'] = p['wk1']
    p['wv1e'] = p['wv1']
    wpack[:C + 1, 0:128] = p.pop('wq1')
    wpack[:C + 1, 128:256] = p.pop('wk1')
    wpack[:C + 1, 256:384] = p.pop('wq2')
    wpack[:C + 1, 384:512] = p.pop('wk2')
    wpack[:C + 1, 512:576] = p.pop('wv1')
    wpack[:C + 1, 576:640] = p.pop('wv2')
    wpack[:C + 1, 640:704] = pr1
    wpack[:C + 1, 704:768] = m2
    wpack[:, 768:896] = np.eye(128, dtype=np.float32)
    wpack[:C, 896:1472] = mtap.transpose(1, 0, 2).reshape(C, 576)
    p['wpack'] = wpack
    # per-core tensors
    x_img = x.reshape(H, W, C).transpose(2, 0, 1)
    xpad = np.zeros((C, H + 2, W + 2), np.float32)
    xpad[:, 1:-1, 1:-1] = x_img
    rpb1 = np.asarray(inputs['rpb1'], np.float32)
    rpb2 = np.asarray(inputs['rpb2'], np.float32)
    percore = []
    for core in range(NCORES):
        t0 = core * Q
        d = {}
        # aux packs per-core xq ([65,Q] at cols 0:Q) and xdw ([C,9Q] at
        # cols Q:Q+9Q, tap-major) into one tensor so xq can be a tiny
        # first DMA piece
        aux = np.zeros((128, Q + 9 * Q), np.float32)
        aux[:C, 0:Q] = x[t0:t0 + Q].T
        aux[C, 0:Q] = 1.0
        for di in range(3):
            for dj in range(3):
                sh = xpad[:, di:di + H, dj:dj + W].reshape(C, N)
                t = di * 3 + dj
                aux[:C, Q + Q * t:Q + Q * t + Q] = sh[:, t0:t0 + Q]
        d['aux'] = aux
        d['b1'] = _build_bias(rpb1, t0)
        d['b2'] = _build_bias(rpb2, t0)
        percore.append(d)
    p = {k: _bf(v) for k, v in p.items()}
    percore = [{k: _bf(v) for k, v in d.items()} for d in percore]
    return p, percore


def _build_program():
    import concourse.bass as bass
    import concourse.bacc as bacc
    import concourse.tile as tile
    from concourse import mybir
    f32 = mybir.dt.float32
    bf16 = mybir.dt.bfloat16
    AF = mybir.ActivationFunctionType

    nc = bacc.Bacc("TRN2", target_bir_lowering=False, debug=False,
                   num_devices=NCORES)

    di = {}
    for name, shape in [
        ('xT', [C + 1, NPAD]), ('aux', [128, 10 * Q]),
        ('wq1e', [C + 1, 128]), ('wk1e', [C + 1, 128]),
        ('wv1e', [C + 1, C]), ('wpack', [128, 1472]),
        ('b1', [128, PTW]), ('b2', [128, PTW]),
    ]:
        di[name] = nc.dram_tensor(name, shape, bf16, kind="ExternalInput")
    out_d = nc.dram_tensor('out', [C, Q], f32, kind="ExternalOutput")
    cc_in = nc.dram_tensor('cc_in', [C, Q], bf16)
    cc_out = nc.dram_tensor('cc_out', [NCORES, C, Q], bf16,
                            addr_space="Shared")

    with tile.TileContext(nc) as tc:
        with (
            tc.tile_pool(name="const", bufs=1) as cpool,
            tc.tile_pool(name="work", bufs=2) as wpool,
            tc.tile_pool(name="ps_big", bufs=3, space="PSUM") as psb,
            tc.tile_pool(name="ps_held", bufs=1, space="PSUM") as psh,
        ):
            # ---- PE clock warm-up (HAM): dummy matmuls on memset data ----
            warm_src = cpool.tile([128, 512], bf16, name='warm_src')
            nc.vector.memset(warm_src[:], 0.125)
            warm_ctr = [0]

            def warm_mm(n):
                if n <= 0:
                    return
                warm_ctr[0] += 1
                ps_dummy = psb.tile([128, 1024], f32,
                                    name=f'ps_dummy{warm_ctr[0]}', tag='big')
                for _ in range(n):
                    nc.tensor.matmul(ps_dummy[:16, :512], warm_src[:, 0:16],
                                     warm_src[:], start=True, stop=True,
                                     skip_group_check=True)

            warm_mm(N_PRE)
            # ---- constant loads ----
            # duplicate wq1 as its own tiny first DMA so the q-proj
            # LDWEIGHTS isn't gated on the full wpack transfer
            wq1_sb = cpool.tile([C + 1, 128], bf16, name='wq1_sb')
            nc.sync.dma_start(wq1_sb[:], di['wq1e'][:])
            xqT = cpool.tile([C + 1, Q], bf16, name='xqT')
            nc.sync.dma_start(xqT[:], di['aux'][0:C + 1, 0:Q])
            # xT in two tiles so k-proj chunk 0 / early v-proj aren't gated
            # on the full transfer (whole-tile dependency)
            wk1_sb = cpool.tile([C + 1, 128], bf16, name='wk1_sb')
            nc.sync.dma_start(wk1_sb[:], di['wk1e'][:])
            xTa = cpool.tile([C + 1, 512], bf16, name='xTa')
            nc.sync.dma_start(xTa[:], di['xT'][:, 0:512])
            wv1_sb = cpool.tile([C + 1, C], bf16, name='wv1_sb')
            nc.sync.dma_start(wv1_sb[:], di['wv1e'][:])
            wpack_sb = cpool.tile([128, 1472], bf16, name='wpack_sb')
            nc.sync.dma_start(wpack_sb[:], di['wpack'][:])
            xTb = cpool.tile([C + 1, NPAD - 512], bf16, name='xTb')
            nc.sync.dma_start(xTb[:], di['xT'][:, 512:NPAD])
            b_sb = {}
            for l in (1, 2):
                b_sb[l] = cpool.tile([128, PTW], bf16, name=f'b{l}_sb')
            # b1 in 3 pieces so chunk 0 can start early
            for s0, s1 in ((0, 4 * GC), (4 * GC, 8 * GC), (8 * GC, PTW)):
                nc.sync.dma_start(b_sb[1][:, s0:s1], di['b1'][:, s0:s1])
            xdw_sb = cpool.tile([C, 9 * Q], bf16, name='xdw_sb')
            nc.sync.dma_start(xdw_sb[:], di['aux'][0:C, Q:10 * Q])
            for s0, s1 in ((0, 6 * GC), (6 * GC, PTW)):
                nc.sync.dma_start(b_sb[2][:, s0:s1], di['b2'][:, s0:s1])
            w_sb = {
                'wq1': wq1_sb[:, :],
                'wk1': wk1_sb[:, :],
                'wq2': wpack_sb[0:C + 1, 256:384],
                'wk2': wpack_sb[0:C + 1, 384:512],
                'wv1': wv1_sb[:, :],
                'wv2': wpack_sb[0:C + 1, 576:640],
                'proj1': wpack_sb[0:C + 1, 640:704],
                'm2p': wpack_sb[0:C + 1, 704:768],
            }
            id_sb = wpack_sb[0:128, 768:896]
            mtap_sb = wpack_sb[0:C, 896:1472]

            # preload exp table early
            dummy = cpool.tile([1, 1], f32, name='dummy')
            nc.vector.memset(dummy[:], 0.0)
            dummy2 = cpool.tile([1, 1], f32, name='dummy2')
            nc.scalar.activation(dummy2[:], dummy[:], AF.Exp)

            x2T = cpool.tile([C + 1, NPAD], bf16, name='x2T')
            nc.vector.memset(x2T[:, N:], 0.0)
            nc.vector.memset(x2T[C:C + 1, :N], 1.0)
            y1T = cpool.tile([C + 1, Q], bf16, name='y1T')
            nc.vector.memset(y1T[C:C + 1, :], 1.0)

            def nat_layer(l, srcT, src_qT, after_pv=None, srcT2=None):
                """srcT: [65, NPAD] AP (or cols 0:512 when srcT2 covers
                512:NPAD); src_qT: [65, Q] AP. -> attnT [65, Q]."""
                wq, wk, wv = w_sb[f'wq{l}'], w_sb[f'wk{l}'], w_sb[f'wv{l}']

                def src(s0, sz):
                    if srcT2 is not None and s0 >= 512:
                        return srcT2[:, s0 - 512:s0 - 512 + sz]
                    return srcT[:, s0:s0 + sz]
                # q proj -> Mq block-diagonal [64, 648]
                ps_q = psb.tile([128, 1024], f32, name='ps_q', tag='big')
                nc.tensor.matmul(ps_q[:, :Q], wq, src_qT, start=True, stop=True)
                Mq = wpool.tile([128, GC], bf16, name='Mq')
                nc.vector.memset(Mq[:], 0.0)
                for h in range(HEADS):
                    nc.vector.tensor_copy(Mq[32 * h:32 * h + 16, Q * h:Q * h + Q],
                                          ps_q[32 * h:32 * h + 16, :Q])
                # k proj chunk 0 first (unblocks S chunk 0), then v proj
                # (keeps the PE busy while k casts run), then k tail
                kT = wpool.tile([128, NPAD], bf16, name='kT')
                nc.vector.memset(kT[:, N:], 0.0)
                VV = wpool.tile([128, NCH * 68], bf16, name='VV')
                VVr = VV[:].rearrange("p (nb g d) -> p nb g d", g=HEADS, d=17)
                nc.vector.memset(VV[:], 0.0)
                nc.vector.memset(VVr[:, :, :, 16:17], 1.0)

                def kproj(s0, sz):
                    ps_k = psb.tile([128, 1024], f32, name='ps_k', tag='big')
                    nc.tensor.matmul(ps_k[:, :sz], wk, src(s0, sz),
                                     start=True, stop=True)
                    nc.vector.tensor_copy(kT[:, s0:s0 + sz], ps_k[:, :sz])

                def vproj(nb):
                    nv = 128 if nb < NCH - 1 else N - 128 * (NCH - 1)
                    ps_v = psb.tile([128, 1024], f32, name='ps_v', tag='big')
                    nc.tensor.matmul(ps_v[:nv, :C],
                                     src(128 * nb, nv),
                                     wv, start=True, stop=True)
                    nc.vector.tensor_copy(
                        VVr[:nv, nb, :, 0:16],
                        ps_v[:nv, :C].rearrange("p (g d) -> p g d", d=16))

                kproj(0, 512)
                for nb in range(NCH):
                    vproj(nb)
                kproj(512, 512)
                kproj(1024, 272)
                # chunk pipeline: S (2 mm) -> exp -> mult -> PV (accum)
                PTr = wpool.tile([128, PTW], bf16, name='PTr')
                PT = wpool.tile([128, PTW], bf16, name='PT')
                ps_o = psh.tile([128, 512], f32, name='ps_o')
                for nb in range(NCH):
                    psS = psb.tile([128, 1024], f32, name='psS', tag='big')
                    kc = kT[:, 128 * nb:128 * nb + 128]
                    nc.tensor.matmul(psS[:, 0:512], kc, Mq[:, 0:512],
                                     start=True, stop=True,
                                     skip_group_check=True)
                    nc.tensor.matmul(psS[:, 512:GC], kc, Mq[:, 512:GC],
                                     start=True, stop=True,
                                     skip_group_check=True)
                    base = GC * nb
                    nc.scalar.activation(PTr[:, base:base + GC],
                                         psS[:, 0:GC], AF.Exp)
                    nc.vector.tensor_mul(PT[:, base:base + GC],
                                         PTr[:, base:base + GC],
                                         b_sb[l][:, base:base + GC])
                    # head-pair PV: one 324-col matmul per pair; out rows
                    # 0:34 (heads 0,1 x query-col blocks) and 64:98 (2,3)
                    for pr in range(2):
                        nc.tensor.matmul(
                            ps_o[64 * pr:64 * pr + 34, 0:2 * Q],
                            VV[:, 68 * nb + 34 * pr:68 * nb + 34 * pr + 34],
                            PT[:, base + 2 * Q * pr:base + 2 * Q * pr + 2 * Q],
                            start=(nb == 0), stop=(nb == NCH - 1),
                            skip_group_check=True, tile_position=(0, 64 * pr))
                if after_pv is not None:
                    after_pv(ps_o)
                # normalize: transpose each pair block -> reciprocal ->
                # scale. After the transpose, channel rows become columns,
                # so the pair layout only changes column indices here.
                o_sbA = wpool.tile([128, Q], bf16, name='o_sbA')
                o_sbB = wpool.tile([128, Q], bf16, name='o_sbB')
                nc.vector.tensor_copy(o_sbA[:], ps_o[:, 0:Q])
                nc.scalar.copy(o_sbB[:], ps_o[:, Q:2 * Q])
                ps_t = psb.tile([128, 2048], bf16, name='ps_t', tag='big')
                nc.tensor.transpose(ps_t[:, 0:128], o_sbA[:, 0:128], id_sb)
                nc.tensor.transpose(ps_t[:34, 256:384], o_sbA[:, 128:Q], id_sb)
                nc.tensor.transpose(ps_t[:, 1024:1152], o_sbB[:, 0:128], id_sb)
                nc.tensor.transpose(ps_t[:34, 1280:1408], o_sbB[:, 128:Q],
                                    id_sb)
                rec = wpool.tile([128, 8], f32, name='rec')
                tA0 = ps_t[:, 0:128].rearrange("p (g d) -> p g d", d=64)
                tB0 = ps_t[:, 1024:1152].rearrange("p (g d) -> p g d", d=64)
                tA1 = ps_t[:34, 256:384].rearrange("p (g d) -> p g d", d=64)
                tB1 = ps_t[:34, 1280:1408].rearrange("p (g d) -> p g d", d=64)
                nc.vector.reciprocal(rec[:, 0:2], tA0[:, :, 16:17])
                nc.vector.reciprocal(rec[:, 2:4], tB0[:, :, 33:34])
                nc.vector.reciprocal(rec[:34, 4:6], tA1[:, :, 16:17])
                nc.vector.reciprocal(rec[:34, 6:8], tB1[:, :, 33:34])
                aq0 = wpool.tile([128, C], bf16, name='aq0')
                aq1 = wpool.tile([34, C], bf16, name='aq1')
                for h in range(HEADS):
                    in_a = (h % 2 == 0)
                    b0 = 0 if in_a else 1024
                    b1 = 256 if in_a else 1280
                    col = 64 * (h // 2) + (0 if in_a else 17)
                    r0 = {0: 0, 2: 1, 1: 2, 3: 3}[h]
                    # split the 8 normalize muls across vector and scalar
                    if h < 2:
                        nc.vector.tensor_scalar_mul(
                            aq0[:, 16 * h:16 * h + 16],
                            ps_t[:, b0 + col:b0 + col + 16],
                            rec[:, r0:r0 + 1])
                        nc.vector.tensor_scalar_mul(
                            aq1[:, 16 * h:16 * h + 16],
                            ps_t[:34, b1 + col:b1 + col + 16],
                            rec[:34, 4 + r0:5 + r0])
                    else:
                        nc.scalar.activation(
                            aq0[:, 16 * h:16 * h + 16],
                            ps_t[:, b0 + col:b0 + col + 16], AF.Copy,
                            scale=rec[:, r0:r0 + 1])
                        nc.scalar.activation(
                            aq1[:, 16 * h:16 * h + 16],
                            ps_t[:34, b1 + col:b1 + col + 16], AF.Copy,
                            scale=rec[:34, 4 + r0:5 + r0])
                ps_a = psb.tile([128, 2048], bf16, name='ps_a', tag='big')
                nc.tensor.transpose(ps_a[:C, 0:128], aq0[:], id_sb)
                nc.tensor.transpose(ps_a[:C, 1024:1058], aq1[:], id_sb[:34, :34])
                # two tiles so consumers of the 128-query half don't wait
                # on the 34-query tail (whole-tile dependency)
                attnTa = wpool.tile([C + 1, 128], bf16, name=f'attnTa{l}')
                attnTb = wpool.tile([C + 1, Q - 128], bf16, name=f'attnTb{l}')
                nc.vector.memset(attnTa[C:C + 1, :], 1.0)
                nc.vector.memset(attnTb[C:C + 1, :], 1.0)
                nc.scalar.copy(attnTa[:C, :], ps_a[:C, 0:128])
                nc.vector.tensor_copy(attnTb[:C, :], ps_a[:C, 1024:1058])
                return attnTa, attnTb

            # ---------------- layer 1 ----------------
            attnT1a, attnT1b = nat_layer(1, xTa[:], xqT[:, :], srcT2=xTb[:])
            ps_y = psb.tile([128, 1024], f32, name='ps_y', tag='big')
            nc.tensor.matmul(ps_y[:C, 0:128], w_sb['proj1'], attnT1a[:],
                             start=True, stop=True, skip_group_check=True)
            nc.scalar.copy(y1T[:C, 0:128], ps_y[:C, 0:128])
            nc.tensor.matmul(ps_y[:C, 128:Q], w_sb['proj1'], attnT1b[:],
                             start=True, stop=True, skip_group_check=True)
            nc.vector.tensor_copy(y1T[:C, 128:Q], ps_y[:C, 128:Q])
            nc.sync.dma_start(cc_in[:], y1T[:C, :])
            nc.gpsimd.collective_compute(
                "AllGather", mybir.AluOpType.bypass,
                replica_groups=[list(range(NCORES))],
                ins=[cc_in.ap().opt()], outs=[cc_out.ap().opt()])
            # dwconv taps overlap the collective
            ps_z = psh.tile([128, 512], f32, name='ps_z')
            for t in range(9):
                nc.tensor.matmul(ps_z[:C, :Q],
                                 mtap_sb[:, C * t:C * t + C],
                                 xdw_sb[:, Q * t:Q * t + Q],
                                 start=(t == 0), stop=False,
                                 skip_group_check=True)
            # keep the PE clock warm through the AllGather bubble
            warm_mm(N_BUB)
            nc.sync.dma_start(x2T[:C, :N],
                              cc_out.ap().rearrange("r c q -> c r q"))
            # ---------------- layer 2 ----------------
            attnT2a, attnT2b = nat_layer(2, x2T[:], y1T[:])
            zo = wpool.tile([C, Q], f32, name='zo')
            nc.tensor.matmul(ps_z[:C, 0:128], w_sb['m2p'], attnT2a[:],
                             start=False, stop=True, skip_group_check=True)
            nc.vector.tensor_copy(zo[:, 0:128], ps_z[:C, 0:128])
            nc.sync.dma_start(out_d[:, 0:128], zo[:, 0:128])
            nc.tensor.matmul(ps_z[:C, 128:Q], w_sb['m2p'], attnT2b[:],
                             start=False, stop=True, skip_group_check=True)
            nc.scalar.copy(zo[:, 128:Q], ps_z[:C, 128:Q])
            nc.sync.dma_start(out_d[:, 128:Q], zo[:, 128:Q])

    nc.finalize()
    return nc


def kernel(**inputs) -> np.ndarray:
    from concourse.bass_utils import run_bass_kernel_spmd
    if 'nc' not in _CACHE:
        _CACHE['nc'] = _build_program()
    nc = _CACHE['nc']
    shared, percore = _prep(inputs)
    in_maps = []
    for core in range(NCORES):
        m = dict(shared)
        m.update(percore[core])
        in_maps.append(m)
    res = run_bass_kernel_spmd(nc, in_maps, core_ids=list(range(NCORES)))
    outs = [np.asarray(res.results[c]['out']).T for c in range(NCORES)]
    full = np.concatenate(outs, axis=0).reshape(1, N, C)
    return full.astype(np.float32)


if __name__ == '__main__':
    import reference
    inputs = reference.setup_inputs()
    inputs = {k: np.asarray(v) for k, v in inputs.items()}
    got = kernel(**inputs)
    print("kernel output", got.shape, got.dtype)



# revision 63
# speedup vs baseline: 1.1761x; 1.0636x over previous
"""Trainium2 Bass kernel for nn_ConvNat (2x NeighborhoodAttention2D + dwconv + linear).

v2 strategy (8 NeuronCores, SPMD), dense-masked attention:
  - Multiplicative bias: host precomputes expB = exp(rpb) in-window, 0 outside.
    P = exp(S) * expB, applied on the vector engine (bf16 2x) -- removes the
    bias identity-matmuls (7128 PE cols/layer) from the tensor engine.
  - Merged-head S matmuls: block-diagonal zero-padded Q [64, 4*Q] so one
    [64,128] stationary computes all 4 heads' S for a key chunk in 2 matmuls
    (512+136 cols) instead of 4.
  - exp in single 648-col slices straight from PSUM.
  - dwconv tap matmuls + layer-2 q-proj run inside the AllGather bubble;
    b1 table DMA'd in 3 pieces so chunk 0 starts early.
"""

import numpy as np
import ml_dtypes

BF16 = ml_dtypes.bfloat16

HEADS = 4
K = 31
C = 64
DH = 16
H = W = 36
N = H * W            # 1296 tokens
NCORES = 8
Q = N // NCORES      # 162 queries per core
NCH = 11             # n-chunks of 128 (1296 -> 1408 padded)
NPAD = NCH * 128
GC = HEADS * Q       # 648 cols per key-chunk block (h, q)
PTW = NCH * GC       # 7128
N_PRE = 0            # PE warm-up dummy matmuls before layer 1
N_BUB = 0            # PE keep-warm dummy matmuls through the AllGather bubble

_CACHE = {}


def _build_bias(rpb, t0):
    """Multiplicative bias exp(rpb) in-window else 0, for queries [t0,t0+Q).
    Layout (128, NCH, HEADS, Q) -> (128, PTW): col = nb*648 + h*162 + q."""
    n = np.arange(NPAD)
    r = np.minimum(n // 36, 35)
    c = n % 36
    valid_n = (n < N)
    t = np.arange(t0, t0 + Q)
    i = t // 36
    j = t % 36
    ri = np.clip(i - K // 2, 0, H - K)
    cj = np.clip(j - K // 2, 0, W - K)
    rm = (r[:, None] >= ri[None, :]) & (r[:, None] <= ri[None, :] + K - 1)
    cm = (c[:, None] >= cj[None, :]) & (c[:, None] <= cj[None, :] + K - 1)
    m = rm & cm & valid_n[:, None]       # (NPAD, Q)
    rrel = np.clip(r[:, None] - i[None, :] + (K - 1), 0, 2 * K - 2)
    crel = np.clip(c[:, None] - j[None, :] + (K - 1), 0, 2 * K - 2)
    bias = rpb[:, rrel, crel]            # (HEADS, NPAD, Q)
    bias = np.where(m[None], np.exp(bias), 0.0).astype(np.float32)
    bias = bias.reshape(HEADS, NCH, 128, Q).transpose(2, 1, 0, 3)
    return np.ascontiguousarray(bias.reshape(128, PTW))


def _interleaved_wk(qkv_w, qkv_b, off, scale=1.0):
    """[65, 128] stationary: cols 32h+0..16 = head-h rows (off+16h+d) of qkv_w^T."""
    wt = np.zeros((C + 1, 128), np.float32)
    for h in range(HEADS):
        rows = qkv_w[off + 16 * h: off + 16 * h + 16]
        wt[:C, 32 * h: 32 * h + 16] = rows.T * scale
        wt[C, 32 * h: 32 * h + 16] = qkv_b[off + 16 * h: off + 16 * h + 16] * scale
    return wt


def _interleaved_wk128(eff_w, eff_b, off, scale=1.0):
    """[128, 128] stationary over the 32-row-per-head gathered layout
    (row 32g+d = channel 16g+d for d<16; row 16 = the ones/bias row)."""
    wt = np.zeros((128, 128), np.float32)
    for kh in range(HEADS):
        rT = eff_w[off + 16 * kh: off + 16 * kh + 16].T  # [C, 16]
        for g in range(HEADS):
            wt[32 * g:32 * g + 16, 32 * kh:32 * kh + 16] = \
                rT[16 * g:16 * g + 16] * scale
        wt[16, 32 * kh:32 * kh + 16] = \
            eff_b[off + 16 * kh: off + 16 * kh + 16] * scale
    return wt


def _wv128(eff_w, eff_b):
    """[128, C] v-projection stationary over the gathered layout."""
    wt = np.zeros((128, C), np.float32)
    wT = eff_w[2 * C:].T  # [C, C]
    for g in range(HEADS):
        wt[32 * g:32 * g + 16] = wT[16 * g:16 * g + 16]
    wt[16] += eff_b[2 * C:]
    return wt


def _bf(a):
    return np.ascontiguousarray(np.asarray(a, np.float32).astype(BF16))


def _prep(inputs):
    x = np.asarray(inputs['x'], np.float32).reshape(N, C)
    p = {}
    xT = np.zeros((C + 1, NPAD), np.float32)
    xT[:C, :N] = x.T
    xT[C, :N] = 1.0
    p['xT'] = xT
    scale = DH ** -0.5
    proj_w1 = np.asarray(inputs['proj_w1'], np.float32)
    proj_b1 = np.asarray(inputs['proj_b1'], np.float32)
    qkv_w1 = np.asarray(inputs['qkv_w1'], np.float32)
    qkv_b1 = np.asarray(inputs['qkv_b1'], np.float32)
    p['wq1'] = _interleaved_wk(qkv_w1, qkv_b1, 0, scale)
    p['wk1'] = _interleaved_wk(qkv_w1, qkv_b1, C)
    wv1 = np.zeros((C + 1, C), np.float32)
    wv1[:C] = qkv_w1[2 * C:].T
    wv1[C] = qkv_b1[2 * C:]
    p['wv1'] = wv1
    # layer-2 qkv with proj1 folded in (y1 = proj1 @ attn1 + proj_b1)
    qkv_w2 = np.asarray(inputs['qkv_w2'], np.float32)
    qkv_b2 = np.asarray(inputs['qkv_b2'], np.float32)
    eff_w2 = qkv_w2 @ proj_w1
    eff_b2 = qkv_w2 @ proj_b1 + qkv_b2
    p['wq2'] = _interleaved_wk(qkv_w2, qkv_b2, 0, scale)
    p['wk2'] = _interleaved_wk(qkv_w2, qkv_b2, C)
    wv2 = np.zeros((C + 1, C), np.float32)
    wv2[:C] = qkv_w2[2 * C:].T
    wv2[C] = qkv_b2[2 * C:]
    p['wv2'] = wv2
    pr1 = np.zeros((C + 1, C), np.float32)
    pr1[:C] = proj_w1.T
    pr1[C] = proj_b1
    lin_w = np.asarray(inputs['lin_w'], np.float32)
    lin_b = np.asarray(inputs['lin_b'], np.float32)
    proj_w2 = np.asarray(inputs['proj_w2'], np.float32)
    proj_b2 = np.asarray(inputs['proj_b2'], np.float32)
    dw_w = np.asarray(inputs['dw_w'], np.float32)               # (64, 1, 3, 3)
    dw_b = np.asarray(inputs['dw_b'], np.float32)
    m2 = np.zeros((C + 1, C), np.float32)
    m2[:C] = (lin_w @ proj_w2).T
    m2[C] = lin_w @ proj_b2 + lin_w @ dw_b + lin_b
    mtap = np.zeros((9, C, C), np.float32)
    for di in range(3):
        for dj in range(3):
            mtap[di * 3 + dj] = (lin_w * dw_w[None, :, 0, di, dj]).T
    # pack small stationary tensors into one [128, 1472] tensor (one DMA)
    wpack = np.zeros((128, 1472), np.float32)
    p['wq1e'] = p['wq1']
    p['wk1e'] = p['wk1']
    p['wv1e'] = p['wv1']
    wpack[:C + 1, 0:128] = p.pop('wq1')
    wpack[:C + 1, 128:256] = p.pop('wk1')
    wpack[:C + 1, 256:384] = p.pop('wq2')
    wpack[:C + 1, 384:512] = p.pop('wk2')
    wpack[:C + 1, 512:576] = p.pop('wv1')
    wpack[:C + 1, 576:640] = p.pop('wv2')
    wpack[:C + 1, 640:704] = pr1
    wpack[:C + 1, 704:768] = m2
    wpack[:, 768:896] = np.eye(128, dtype=np.float32)
    wpack[:C, 896:1472] = mtap.transpose(1, 0, 2).reshape(C, 576)
    p['wpack'] = wpack
    # per-core tensors
    x_img = x.reshape(H, W, C).transpose(2, 0, 1)
    xpad = np.zeros((C, H + 2, W + 2), np.float32)
    xpad[:, 1:-1, 1:-1] = x_img
    rpb1 = np.asarray(inputs['rpb1'], np.float32)
    rpb2 = np.asarray(inputs['rpb2'], np.float32)
    percore = []
    for core in range(NCORES):
        t0 = core * Q
        d = {}
        # aux packs per-core xq ([65,Q] at cols 0:Q) and xdw ([C,9Q] at
        # cols Q:Q+9Q, tap-major) into one tensor so xq can be a tiny
        # first DMA piece
        aux = np.zeros((128, Q + 9 * Q), np.float32)
        aux[:C, 0:Q] = x[t0:t0 + Q].T
        aux[C, 0:Q] = 1.0
        for di in range(3):
            for dj in range(3):
                sh = xpad[:, di:di + H, dj:dj + W].reshape(C, N)
                t = di * 3 + dj
                aux[:C, Q + Q * t:Q + Q * t + Q] = sh[:, t0:t0 + Q]
        d['aux'] = aux
        d['b1'] = _build_bias(rpb1, t0)
        d['b2'] = _build_bias(rpb2, t0)
        percore.append(d)
    p = {k: _bf(v) for k, v in p.items()}
    percore = [{k: _bf(v) for k, v in d.items()} for d in percore]
    return p, percore


def _build_program():
    import concourse.bass as bass
    import concourse.bacc as bacc
    import concourse.tile as tile
    from concourse import mybir
    f32 = mybir.dt.float32
    bf16 = mybir.dt.bfloat16
    AF = mybir.ActivationFunctionType

    nc = bacc.Bacc("TRN2", target_bir_lowering=False, debug=False,
                   num_devices=NCORES)

    di = {}
    for name, shape in [
        ('xT', [C + 1, NPAD]), ('aux', [128, 10 * Q]),
        ('wq1e', [C + 1, 128]), ('wk1e', [C + 1, 128]),
        ('wv1e', [C + 1, C]), ('wpack', [128, 1472]),
        ('b1', [128, PTW]), ('b2', [128, PTW]),
    ]:
        di[name] = nc.dram_tensor(name, shape, bf16, kind="ExternalInput")
    out_d = nc.dram_tensor('out', [C, Q], f32, kind="ExternalOutput")
    cc_in = nc.dram_tensor('cc_in', [C, Q], bf16)
    cc_out = nc.dram_tensor('cc_out', [NCORES, C, Q], bf16,
                            addr_space="Shared")

    with tile.TileContext(nc) as tc:
        with (
            tc.tile_pool(name="const", bufs=1) as cpool,
            tc.tile_pool(name="work", bufs=2) as wpool,
            tc.tile_pool(name="ps_big", bufs=3, space="PSUM") as psb,
            tc.tile_pool(name="ps_held", bufs=1, space="PSUM") as psh,
        ):
            # ---- PE clock warm-up (HAM): dummy matmuls on memset data ----
            warm_src = cpool.tile([128, 512], bf16, name='warm_src')
            nc.vector.memset(warm_src[:], 0.125)
            warm_ctr = [0]

            def warm_mm(n):
                if n <= 0:
                    return
                warm_ctr[0] += 1
                ps_dummy = psb.tile([128, 1024], f32,
                                    name=f'ps_dummy{warm_ctr[0]}', tag='big')
                for _ in range(n):
                    nc.tensor.matmul(ps_dummy[:16, :512], warm_src[:, 0:16],
                                     warm_src[:], start=True, stop=True,
                                     skip_group_check=True)

            warm_mm(N_PRE)
            # ---- constant loads ----
            # duplicate wq1 as its own tiny first DMA so the q-proj
            # LDWEIGHTS isn't gated on the full wpack transfer
            wq1_sb = cpool.tile([C + 1, 128], bf16, name='wq1_sb')
            nc.sync.dma_start(wq1_sb[:], di['wq1e'][:])
            xqT = cpool.tile([C + 1, Q], bf16, name='xqT')
            nc.sync.dma_start(xqT[:], di['aux'][0:C + 1, 0:Q])
            # xT in two tiles so k-proj chunk 0 / early v-proj aren't gated
            # on the full transfer (whole-tile dependency)
            wk1_sb = cpool.tile([C + 1, 128], bf16, name='wk1_sb')
            nc.sync.dma_start(wk1_sb[:], di['wk1e'][:])
            xTa = cpool.tile([C + 1, 512], bf16, name='xTa')
            nc.sync.dma_start(xTa[:], di['xT'][:, 0:512])
            wv1_sb = cpool.tile([C + 1, C], bf16, name='wv1_sb')
            nc.sync.dma_start(wv1_sb[:], di['wv1e'][:])
            wpack_sb = cpool.tile([128, 1472], bf16, name='wpack_sb')
            nc.sync.dma_start(wpack_sb[:], di['wpack'][:])
            xTb = cpool.tile([C + 1, NPAD - 512], bf16, name='xTb')
            nc.sync.dma_start(xTb[:], di['xT'][:, 512:NPAD])
            b_sb = {}
            for l in (1, 2):
                b_sb[l] = cpool.tile([128, PTW], bf16, name=f'b{l}_sb')
            # b1 in 3 pieces so chunk 0 can start early
            for s0, s1 in ((0, 4 * GC), (4 * GC, 8 * GC), (8 * GC, PTW)):
                nc.sync.dma_start(b_sb[1][:, s0:s1], di['b1'][:, s0:s1])
            xdw_sb = cpool.tile([C, 9 * Q], bf16, name='xdw_sb')
            nc.sync.dma_start(xdw_sb[:], di['aux'][0:C, Q:10 * Q])
            for s0, s1 in ((0, 6 * GC), (6 * GC, PTW)):
                nc.sync.dma_start(b_sb[2][:, s0:s1], di['b2'][:, s0:s1])
            w_sb = {
                'wq1': wq1_sb[:, :],
                'wk1': wk1_sb[:, :],
                'wq2': wpack_sb[0:C + 1, 256:384],
                'wk2': wpack_sb[0:C + 1, 384:512],
                'wv1': wv1_sb[:, :],
                'wv2': wpack_sb[0:C + 1, 576:640],
                'proj1': wpack_sb[0:C + 1, 640:704],
                'm2p': wpack_sb[0:C + 1, 704:768],
            }
            id_sb = wpack_sb[0:128, 768:896]
            mtap_sb = wpack_sb[0:C, 896:1472]

            # preload exp table early
            dummy = cpool.tile([1, 1], f32, name='dummy')
            nc.vector.memset(dummy[:], 0.0)
            dummy2 = cpool.tile([1, 1], f32, name='dummy2')
            nc.scalar.activation(dummy2[:], dummy[:], AF.Exp)

            x2T = cpool.tile([C + 1, NPAD], bf16, name='x2T')
            nc.vector.memset(x2T[:, N:], 0.0)
            nc.vector.memset(x2T[C:C + 1, :N], 1.0)
            y1T = cpool.tile([C + 1, Q], bf16, name='y1T')
            nc.vector.memset(y1T[C:C + 1, :], 1.0)

            def nat_layer(l, srcT, src_qT, after_pv=None, srcT2=None):
                """srcT: [65, NPAD] AP (or cols 0:512 when srcT2 covers
                512:NPAD); src_qT: [65, Q] AP. -> attnT [65, Q]."""
                wq, wk, wv = w_sb[f'wq{l}'], w_sb[f'wk{l}'], w_sb[f'wv{l}']

                def src(s0, sz):
                    if srcT2 is not None and s0 >= 512:
                        return srcT2[:, s0 - 512:s0 - 512 + sz]
                    return srcT[:, s0:s0 + sz]
                # q proj -> Mq block-diagonal [64, 648]
                ps_q = psb.tile([128, 1024], f32, name='ps_q', tag='big')
                nc.tensor.matmul(ps_q[:, :Q], wq, src_qT, start=True, stop=True)
                Mq = wpool.tile([128, GC], bf16, name='Mq')
                nc.vector.memset(Mq[:], 0.0)
                for h in range(HEADS):
                    nc.vector.tensor_copy(Mq[32 * h:32 * h + 16, Q * h:Q * h + Q],
                                          ps_q[32 * h:32 * h + 16, :Q])
                # k proj chunk 0 first (unblocks S chunk 0), then v proj
                # (keeps the PE busy while k casts run), then k tail
                kT = wpool.tile([128, NPAD], bf16, name='kT')
                nc.vector.memset(kT[:, N:], 0.0)
                VV = wpool.tile([128, NCH * 68], bf16, name='VV')
                VVr = VV[:].rearrange("p (nb g d) -> p nb g d", g=HEADS, d=17)
                nc.vector.memset(VV[:], 0.0)
                nc.vector.memset(VVr[:, :, :, 16:17], 1.0)

                def kproj(s0, sz):
                    ps_k = psb.tile([128, 1024], f32, name='ps_k', tag='big')
                    nc.tensor.matmul(ps_k[:, :sz], wk, src(s0, sz),
                                     start=True, stop=True)
                    nc.vector.tensor_copy(kT[:, s0:s0 + sz], ps_k[:, :sz])

                def vproj(nb):
                    nv = 128 if nb < NCH - 1 else N - 128 * (NCH - 1)
                    ps_v = psb.tile([128, 1024], f32, name='ps_v', tag='big')
                    nc.tensor.matmul(ps_v[:nv, :C],
                                     src(128 * nb, nv),
                                     wv, start=True, stop=True)
                    nc.vector.tensor_copy(
                        VVr[:nv, nb, :, 0:16],
                        ps_v[:nv, :C].rearrange("p (g d) -> p g d", d=16))

                kproj(0, 512)
                for nb in range(NCH):
                    vproj(nb)
                kproj(512, 512)
                kproj(1024, 272)
                # chunk pipeline: S (2 mm) -> exp -> mult -> PV (accum)
                PTr = wpool.tile([128, PTW], bf16, name='PTr')
                PT = wpool.tile([128, PTW], bf16, name='PT')
                ps_o = psh.tile([128, 512], f32, name='ps_o')
                for nb in range(NCH):
                    psS = psb.tile([128, 1024], f32, name='psS', tag='big')
                    kc = kT[:, 128 * nb:128 * nb + 128]
                    nc.tensor.matmul(psS[:, 0:512], kc, Mq[:, 0:512],
                                     start=True, stop=True,
                                     skip_group_check=True)
                    nc.tensor.matmul(psS[:, 512:GC], kc, Mq[:, 512:GC],
                                     start=True, stop=True,
                                     skip_group_check=True)
                    base = GC * nb
                    nc.scalar.activation(PTr[:, base:base + GC],
                                         psS[:, 0:GC], AF.Exp)
                    nc.vector.tensor_mul(PT[:, base:base + GC],
                                         PTr[:, base:base + GC],
                                         b_sb[l][:, base:base + GC])
                    # head-pair PV: one 324-col matmul per pair; out rows
                    # 0:34 (heads 0,1 x query-col blocks) and 64:98 (2,3)
                    for pr in range(2):
                        nc.tensor.matmul(
                            ps_o[64 * pr:64 * pr + 34, 0:2 * Q],
                            VV[:, 68 * nb + 34 * pr:68 * nb + 34 * pr + 34],
                            PT[:, base + 2 * Q * pr:base + 2 * Q * pr + 2 * Q],
                            start=(nb == 0), stop=(nb == NCH - 1),
                            skip_group_check=True, tile_position=(0, 64 * pr))
                if after_pv is not None:
                    after_pv(ps_o)
                # normalize: transpose each pair block -> reciprocal ->
                # scale. After the transpose, channel rows become columns,
                # so the pair layout only changes column indices here.
                o_sbA = wpool.tile([128, Q], bf16, name='o_sbA')
                o_sbB = wpool.tile([128, Q], bf16, name='o_sbB')
                nc.vector.tensor_copy(o_sbA[:], ps_o[:, 0:Q])
                nc.scalar.copy(o_sbB[:], ps_o[:, Q:2 * Q])
                ps_t = psb.tile([128, 2048], bf16, name='ps_t', tag='big')
                nc.tensor.transpose(ps_t[:, 0:128], o_sbA[:, 0:128], id_sb)
                nc.tensor.transpose(ps_t[:34, 256:384], o_sbA[:, 128:Q], id_sb)
                nc.tensor.transpose(ps_t[:, 1024:1152], o_sbB[:, 0:128], id_sb)
                nc.tensor.transpose(ps_t[:34, 1280:1408], o_sbB[:, 128:Q],
                                    id_sb)
                rec = wpool.tile([128, 8], f32, name='rec')
                tA0 = ps_t[:, 0:128].rearrange("p (g d) -> p g d", d=64)
                tB0 = ps_t[:, 1024:1152].rearrange("p (g d) -> p g d", d=64)
                tA1 = ps_t[:34, 256:384].rearrange("p (g d) -> p g d", d=64)
                tB1 = ps_t[:34, 1280:1408].rearrange("p (g d) -> p g d", d=64)
                nc.vector.reciprocal(rec[:, 0:2], tA0[:, :, 16:17])
                nc.vector.reciprocal(rec[:, 2:4], tB0[:, :, 33:34])
                nc.vector.reciprocal(rec[:34, 4:6], tA1[:, :, 16:17])
                nc.vector.reciprocal(rec[:34, 6:8], tB1[:, :, 33:34])
                # per-engine aq tiles: a shared tile would serialize the
                # scalar muls behind all vector muls (whole-tile writers)
                aq0v = wpool.tile([128, 32], bf16, name='aq0v')
                aq0s = wpool.tile([128, 32], bf16, name='aq0s')
                aq1v = wpool.tile([34, 32], bf16, name='aq1v')
                aq1s = wpool.tile([34, 32], bf16, name='aq1s')
                for h in range(HEADS):
                    in_a = (h % 2 == 0)
                    b0 = 0 if in_a else 1024
                    b1 = 256 if in_a else 1280
                    col = 64 * (h // 2) + (0 if in_a else 17)
                    r0 = {0: 0, 2: 1, 1: 2, 3: 3}[h]
                    if h < 2:
                        nc.vector.tensor_scalar_mul(
                            aq0v[:, 16 * h:16 * h + 16],
                            ps_t[:, b0 + col:b0 + col + 16],
                            rec[:, r0:r0 + 1])
                        nc.vector.tensor_scalar_mul(
                            aq1v[:, 16 * h:16 * h + 16],
                            ps_t[:34, b1 + col:b1 + col + 16],
                            rec[:34, 4 + r0:5 + r0])
                    else:
                        nc.scalar.activation(
                            aq0s[:, 16 * (h - 2):16 * (h - 2) + 16],
                            ps_t[:, b0 + col:b0 + col + 16], AF.Copy,
                            scale=rec[:, r0:r0 + 1])
                        nc.scalar.activation(
                            aq1s[:, 16 * (h - 2):16 * (h - 2) + 16],
                            ps_t[:34, b1 + col:b1 + col + 16], AF.Copy,
                            scale=rec[:34, 4 + r0:5 + r0])
                ps_a = psb.tile([128, 2048], bf16, name='ps_a', tag='big')
                nc.tensor.transpose(ps_a[0:32, 0:128], aq0v[:], id_sb)
                nc.tensor.transpose(ps_a[32:64, 0:128], aq0s[:], id_sb)
                nc.tensor.transpose(ps_a[0:32, 1024:1058], aq1v[:],
                                    id_sb[:34, :34])
                nc.tensor.transpose(ps_a[32:64, 1024:1058], aq1s[:],
                                    id_sb[:34, :34])
                # two tiles so consumers of the 128-query half don't wait
                # on the 34-query tail (whole-tile dependency)
                attnTa = wpool.tile([C + 1, 128], bf16, name=f'attnTa{l}')
                attnTb = wpool.tile([C + 1, Q - 128], bf16, name=f'attnTb{l}')
                nc.vector.memset(attnTa[C:C + 1, :], 1.0)
                nc.vector.memset(attnTb[C:C + 1, :], 1.0)
                nc.scalar.copy(attnTa[:C, :], ps_a[:C, 0:128])
                nc.vector.tensor_copy(attnTb[:C, :], ps_a[:C, 1024:1058])
                return attnTa, attnTb

            # ---------------- layer 1 ----------------
            attnT1a, attnT1b = nat_layer(1, xTa[:], xqT[:, :], srcT2=xTb[:])
            ps_y = psb.tile([128, 1024], f32, name='ps_y', tag='big')
            nc.tensor.matmul(ps_y[:C, 0:128], w_sb['proj1'], attnT1a[:],
                             start=True, stop=True, skip_group_check=True)
            nc.scalar.copy(y1T[:C, 0:128], ps_y[:C, 0:128])
            nc.tensor.matmul(ps_y[:C, 128:Q], w_sb['proj1'], attnT1b[:],
                             start=True, stop=True, skip_group_check=True)
            nc.vector.tensor_copy(y1T[:C, 128:Q], ps_y[:C, 128:Q])
            nc.sync.dma_start(cc_in[:], y1T[:C, :])
            nc.gpsimd.collective_compute(
                "AllGather", mybir.AluOpType.bypass,
                replica_groups=[list(range(NCORES))],
                ins=[cc_in.ap().opt()], outs=[cc_out.ap().opt()])
            # dwconv taps overlap the collective
            ps_z = psh.tile([128, 512], f32, name='ps_z')
            for t in range(9):
                nc.tensor.matmul(ps_z[:C, :Q],
                                 mtap_sb[:, C * t:C * t + C],
                                 xdw_sb[:, Q * t:Q * t + Q],
                                 start=(t == 0), stop=False,
                                 skip_group_check=True)
            # keep the PE clock warm through the AllGather bubble
            warm_mm(N_BUB)
            nc.sync.dma_start(x2T[:C, :N],
                              cc_out.ap().rearrange("r c q -> c r q"))
            # ---------------- layer 2 ----------------
            attnT2a, attnT2b = nat_layer(2, x2T[:], y1T[:])
            zo = wpool.tile([C, Q], f32, name='zo')
            nc.tensor.matmul(ps_z[:C, 0:128], w_sb['m2p'], attnT2a[:],
                             start=False, stop=True, skip_group_check=True)
            nc.vector.tensor_copy(zo[:, 0:128], ps_z[:C, 0:128])
            nc.sync.dma_start(out_d[:, 0:128], zo[:, 0:128])
            nc.tensor.matmul(ps_z[:C, 128:Q], w_sb['m2p'], attnT2b[:],
                             start=False, stop=True, skip_group_check=True)
            nc.scalar.copy(zo[:, 128:Q], ps_z[:C, 128:Q])
            nc.sync.dma_start(out_d[:, 128:Q], zo[:, 128:Q])

    nc.finalize()
    return nc


def kernel(**inputs) -> np.ndarray:
    from concourse.bass_utils import run_bass_kernel_spmd
    if 'nc' not in _CACHE:
        _CACHE['nc'] = _build_program()
    nc = _CACHE['nc']
    shared, percore = _prep(inputs)
    in_maps = []
    for core in range(NCORES):
        m = dict(shared)
        m.update(percore[core])
        in_maps.append(m)
    res = run_bass_kernel_spmd(nc, in_maps, core_ids=list(range(NCORES)))
    outs = [np.asarray(res.results[c]['out']).T for c in range(NCORES)]
    full = np.concatenate(outs, axis=0).reshape(1, N, C)
    return full.astype(np.float32)


if __name__ == '__main__':
    import reference
    inputs = reference.setup_inputs()
    inputs = {k: np.asarray(v) for k, v in inputs.items()}
    got = kernel(**inputs)
    print("kernel output", got.shape, got.dtype)

